# revision 14
# baseline (speedup 1.0000x reference)
"""GQA attention kernel for Trainium2, tensor-parallel over heads across 8 cores.

Problem: T=2048, D=4096, H=32 q-heads, G=8 kv-heads, HD=128.
Per core: 4 q heads + 1 kv head (group), full T. All matmuls bf16.

Single interleaved pass over 16 token blocks:
  per block: q/k/v projections (bf16 in, f32 psum) -> rmsnorm (accum_out
  squares) -> rope (merged DVE ops over pre-tiled tables) -> bf16 qT/kT
  via transpose-by-matmul (identity moving operand) -> v bf16.
  After each group of 4 blocks (supertile of 512 queries): attention for
  that supertile: j-outer loop, scores for 4 heads sharing the kT_j
  stationary, exp -> bf16 ex, A@V + ones-matmul denominators (4 heads
  partition-packed into one psum bank), normalize, then out-proj
  (row-sharded Wo) and DMA of the 4 output row blocks.
Host sums the 8 partial outputs.
"""
import sys

sys.path.insert(0, '/opt/trn_rl_repo')

import numpy as np
import ml_dtypes

import concourse.bass as bass
import concourse.bacc as bacc
import concourse.mybir as mybir
import concourse.tile as tile
from concourse.bass_utils import run_bass_kernel_spmd

F32 = mybir.dt.float32
BF16 = mybir.dt.bfloat16
AF = mybir.ActivationFunctionType
OP = mybir.AluOpType

T = 2048
D = 4096
H = 32
G = 8
HD = 128
NCORES = 8
HPC = H // NCORES          # 4 q heads per core
NB = T // 128              # 16 row/col blocks
NSUP = NB // 4             # 4 supertiles of 512 queries
DKT = D // 128             # 32 contraction tiles for projections
NOC = D // 512             # 8 out-proj column blocks
EPS = 1e-6
ISQ = 1.0 / float(np.sqrt(HD))


def _rotview4(ap):
    """[128, 512] AP -> [128, 4, 2, 64] view; per 128-chunk reads cols
    64:128 then 0:64 (rotate-half source for rope)."""
    return bass.AP(ap.tensor, ap.offset + 64,
                   [list(ap.ap[0]), [128, 4], [-64, 2], [1, 64]])


def _rotview1(ap):
    """[128, 128] AP -> [128, 2, 64] view reading cols 64:128 then 0:64."""
    return bass.AP(ap.tensor, ap.offset + 64, [list(ap.ap[0]), [-64, 2], [1, 64]])


def _emit(nc, tc):
    xt = nc.dram_tensor("xt", [NB, 128, DKT * 128], BF16, kind="ExternalInput")
    wq = nc.dram_tensor("wq", [128, DKT * 512], BF16, kind="ExternalInput")
    wkv = nc.dram_tensor("wkv", [128, DKT * 256], BF16, kind="ExternalInput")
    wo = nc.dram_tensor("wo", [128, HPC * NOC * 512], BF16, kind="ExternalInput")
    # per token: [cq x4 (512) | sq x4 (512) | ck (128) | sk (128)]
    tbl = nc.dram_tensor("tbl", [T, 1280], BF16, kind="ExternalInput")
    tri01 = nc.dram_tensor("tri01", [128, 128], F32, kind="ExternalInput")
    ident = nc.dram_tensor("ident", [128, 128], BF16, kind="ExternalInput")
    onescol = nc.dram_tensor("onescol", [128, 1], BF16, kind="ExternalInput")
    out = nc.dram_tensor("out", [T, D], F32, kind="ExternalOutput")

    import contextlib
    ctx = contextlib.ExitStack()
    with ctx:
        const_p = ctx.enter_context(tc.tile_pool(name="const", bufs=1))
        tri_sb = const_p.tile([128, 128], F32)
        id_sb = const_p.tile([128, 128], BF16)
        ones_sb = const_p.tile([128, 1], BF16)
        epsb = const_p.tile([128, 1], F32)

        pers = ctx.enter_context(tc.tile_pool(name="pers", bufs=1))
        wq_sb = pers.tile([128, DKT * 512], BF16)
        wkv_sb = pers.tile([128, DKT * 256], BF16)
        wo_sb = pers.tile([128, HPC * NOC * 512], BF16)
        kT = pers.tile([128, T], BF16)
        v_sb = pers.tile([128, NB, 128], BF16)

        wq3 = wq_sb[:].rearrange("p (k n) -> p k n", k=DKT)
        wkv3 = wkv_sb[:].rearrange("p (k n) -> p k n", k=DKT)
        wo4 = wo_sb[:].rearrange("p (h n c) -> p h n c", h=HPC, n=NOC)

        xb_p = ctx.enter_context(tc.tile_pool(name="xb", bufs=2))
        tbl_p = ctx.enter_context(tc.tile_pool(name="tblp", bufs=3))
        scr_p = ctx.enter_context(tc.tile_pool(name="scr", bufs=2))
        rope_p = ctx.enter_context(tc.tile_pool(name="rope", bufs=2))
        qt_p = ctx.enter_context(tc.tile_pool(name="qt", bufs=2))
        at_p = ctx.enter_context(tc.tile_pool(name="at", bufs=2))
        ex_p = ctx.enter_context(tc.tile_pool(name="exp", bufs=10))
        rcb_p = ctx.enter_context(tc.tile_pool(name="rcb", bufs=2))
        ost_p = ctx.enter_context(tc.tile_pool(name="ost", bufs=2))

        # PSUM: 8 banks:
        #   ps_a (3): proj-q psum / q transposes / score tiles  [128,512]
        #   ps_c (2): proj-kv psum (1) + denominators (1)
        #   ps_pat (3): A@V accumulators / k transpose / out-proj
        ps_a = ctx.enter_context(
            tc.tile_pool(name="ps_a", bufs=3, space="PSUM", side="right"))
        ps_c = ctx.enter_context(tc.tile_pool(name="ps_c", bufs=1, space="PSUM"))
        ps_pat = ctx.enter_context(
            tc.tile_pool(name="ps_pat", bufs=2, space="PSUM", side="right"))

        dmabuf = {}

        def dma_block(i):
            xb = xb_p.tile([128, DKT * 128], BF16, tag="xb", name=f"xb{i}")
            nch = 8 if i == 0 else 2
            for ch in range(nch):
                w = DKT * 128 // nch
                nc.sync.dma_start(xb[:, ch * w:(ch + 1) * w],
                                  xt.ap()[i][:, ch * w:(ch + 1) * w])
            tb = tbl_p.tile([128, 1280], BF16, tag="tb", name=f"tb{i}")
            nc.sync.dma_start(tb[:], tbl.ap()[i * 128:(i + 1) * 128, :])
            dmabuf[i] = (xb, tb)

        def load_w_initial():
            nc.gpsimd.dma_start(wq_sb[:, 0:2 * 512], wq.ap()[:, 0:2 * 512])
            nc.sync.dma_start(wkv_sb[:, 0:2 * 256], wkv.ap()[:, 0:2 * 256])
            nc.scalar.dma_start(wq_sb[:, 2 * 512:4 * 512],
                                wq.ap()[:, 2 * 512:4 * 512])
            nc.scalar.dma_start(wkv_sb[:, 2 * 256:4 * 256],
                                wkv.ap()[:, 2 * 256:4 * 256])
            nc.gpsimd.dma_start(id_sb[:], ident.ap())
            nc.gpsimd.dma_start(tri_sb[:], tri01.ap())
            nc.gpsimd.dma_start(ones_sb[:], onescol.ap())
            nc.vector.memset(epsb[:], float(HD) * EPS)

        def load_w_rest():
            # spread across 3 engine queues, ktile-ordered so block-0 matmuls
            # can stream behind the transfers
            engs = [nc.gpsimd, nc.sync, nc.scalar]
            for idx, c in enumerate(range(4, 32, 2)):
                e = c + 2
                eng = engs[idx % 3]
                eng.dma_start(wq_sb[:, c * 512:e * 512],
                              wq.ap()[:, c * 512:e * 512])
                eng.dma_start(wkv_sb[:, c * 256:e * 256],
                              wkv.ap()[:, c * 256:e * 256])

        def load_wo(stage):
            w = HPC * NOC * 512 // 4
            nc.gpsimd.dma_start(wo_sb[:, stage * w:(stage + 1) * w],
                                wo.ap()[:, stage * w:(stage + 1) * w])

        psbuf = {}

        def proj_mms(i):
            xb, _ = dmabuf[i]
            xb3 = xb[:].rearrange("p (k n) -> p k n", k=DKT)
            psq = ps_a.tile([128, 512], F32, tag="psa", name=f"psq{i}")
            pskv = ps_c.tile([128, 512], F32, tag="psc", name=f"pskv{i}")
            for kk in range(DKT):
                nc.tensor.matmul(psq[:], xb3[:, kk, :], wq3[:, kk, :],
                                 start=(kk == 0), stop=(kk == DKT - 1),
                                 skip_group_check=True)
                nc.tensor.matmul(pskv[:, 0:256], xb3[:, kk, :], wkv3[:, kk, :],
                                 start=(kk == 0), stop=(kk == DKT - 1),
                                 skip_group_check=True)
            psbuf[i] = (psq, pskv)

        qt_hold = [None]
        ropebuf = {}

        def rms_rope(i):
            psq, pskv = psbuf.pop(i)
            _, tb = dmabuf.pop(i)
            cqR, sqR = tb[:, 0:512], tb[:, 512:1024]
            ck, sk = tb[:, 1024:1152], tb[:, 1152:1280]

            # rms stats for 4 q chunks + 1 k chunk
            ssq = scr_p.tile([128, 8], F32, tag="ssq")
            sqscr = scr_p.tile([128, 128], F32, tag="sqscr")
            for c in range(HPC):
                nc.scalar.activation(sqscr[:], psq[:, c * 128:(c + 1) * 128],
                                     AF.Square, accum_out=ssq[:, c:c + 1])
            nc.scalar.activation(sqscr[:], pskv[:, 0:128],
                                 AF.Square, accum_out=ssq[:, 4:5])
            rstd = scr_p.tile([128, 8], F32, tag="rstd")
            nc.scalar.activation(rstd[:, 0:5], ssq[:, 0:5], AF.Sqrt, bias=epsb[:])
            nc.vector.reciprocal_approx_fast(rstd[:, 0:5], rstd[:, 0:5])

            # v: psum -> sbuf bf16
            nc.scalar.copy(v_sb[:, i, :], pskv[:, 128:256])

            # q: scale by rstd on scalar engine (per chunk), then merged rope
            qs = rope_p.tile([128, 512], BF16, tag="qs")
            for c in range(HPC):
                nc.scalar.activation(qs[:, c * 128:(c + 1) * 128],
                                     psq[:, c * 128:(c + 1) * 128],
                                     AF.Copy, scale=rstd[:, c:c + 1])
            t1 = rope_p.tile([128, 512], F32, tag="t1")
            t2 = rope_p.tile([128, 512], F32, tag="t2")
            roq = rope_p.tile([128, 512], BF16, tag="roq")
            nc.vector.tensor_tensor(t1[:], qs[:], cqR, op=OP.mult)
            nc.vector.tensor_tensor(
                t2[:].rearrange("p (c a b) -> p c a b", c=4, a=2),
                _rotview4(qs[:]),
                sqR.rearrange("p (c a b) -> p c a b", c=4, a=2), op=OP.mult)
            nc.vector.tensor_add(roq[:], t1[:], t2[:])

            # k rope (rstd fused via scalar_tensor_tensor)
            rok = rope_p.tile([128, 128], BF16, tag="rok")
            kt1 = scr_p.tile([128, 128], F32, tag="kt1")
            kt2 = scr_p.tile([128, 128], F32, tag="kt2")
            chk = pskv[:, 0:128]
            nc.vector.scalar_tensor_tensor(kt1[:], chk, rstd[:, 4:5], ck,
                                           op0=OP.mult, op1=OP.mult)
            nc.vector.scalar_tensor_tensor(
                kt2[:].rearrange("p (a b) -> p a b", a=2), _rotview1(chk),
                rstd[:, 4:5], sk.rearrange("p (a b) -> p a b", a=2),
                op0=OP.mult, op1=OP.mult)
            nc.vector.tensor_add(rok[:], kt1[:], kt2[:])
            ropebuf[i] = (roq, rok)

        def transp_block(i):
            roq, rok = ropebuf.pop(i)
            if i % 4 == 0:
                qt_hold[0] = qt_p.tile([128, HPC, 512], BF16, tag="qt",
                                       name=f"qt{i // 4}")
            qtile = qt_hold[0]
            ib = (i % 4) * 128
            # transposes via matmul with identity moving operand
            trq = ps_a.tile([128, 512], F32, tag="psa", name=f"trq{i}")
            for c in range(HPC):
                nc.tensor.matmul(trq[:, c * 128:(c + 1) * 128],
                                 roq[:, c * 128:(c + 1) * 128], id_sb[:],
                                 start=True, stop=True)
            trk = ps_pat.tile([128, 512], F32, tag="pat", name=f"trk{i}")
            nc.tensor.matmul(trk[:, 0:128], rok[:], id_sb[:],
                             start=True, stop=True)
            for c in range(HPC):
                if c % 2 == 0:
                    nc.scalar.copy(qtile[:, c, ib:ib + 128],
                                   trq[:, c * 128:(c + 1) * 128])
                else:
                    nc.vector.tensor_copy(qtile[:, c, ib:ib + 128],
                                          trq[:, c * 128:(c + 1) * 128])
            nc.vector.tensor_copy(kT[:, i * 128:(i + 1) * 128], trk[:, 0:128])

        # ---------------- attention + out-proj for one supertile -----------
        def attn_supertile(g):
            qtile = qt_hold[0]
            nj = 4 * g + 4
            atile = at_p.tile([128, HPC, 512], BF16, tag="at", name=f"at{g}")

            for hp in range(HPC // 2):
                hs = (2 * hp, 2 * hp + 1)
                pats = {h: ps_pat.tile([128, 512], F32, tag="pat",
                                       name=f"pat{g}_{h}") for h in hs}
                pdens = {h: ps_c.tile([128, 512], F32, tag="pden", bufs=2,
                                      name=f"pden{g}_{h}") for h in hs}

                def av_block(j, lo, exs):
                    first, last = (j == 0), (j == nj - 1)
                    for h in hs:
                        nc.tensor.matmul(pats[h][:, lo:512], v_sb[:, j, :],
                                         exs[h][:, lo:512],
                                         start=first, stop=last,
                                         skip_group_check=True)
                    for h in hs:
                        nc.tensor.matmul(pdens[h][0:1, lo:512], ones_sb[:],
                                         exs[h][:, lo:512],
                                         start=first, stop=last,
                                         skip_group_check=True)

                pend = None
                for j in range(nj):
                    cd = j - 4 * g
                    lo = max(0, cd) * 128
                    exs = {}
                    for h in hs:
                        psc = ps_a.tile([128, 512], F32, tag="psa",
                                        name=f"psc{g}_{j}_{h}")
                        nc.tensor.matmul(psc[:, lo:512],
                                         kT[:, j * 128:(j + 1) * 128],
                                         qtile[:, h, lo:512],
                                         start=True, stop=True)
                        ex = ex_p.tile([128, 512], BF16, tag="ex")
                        nc.scalar.activation(ex[:, lo:512], psc[:, lo:512],
                                             AF.Exp, scale=ISQ)
                        if 0 <= cd <= 3:
                            nc.vector.tensor_mul(ex[:, cd * 128:(cd + 1) * 128],
                                                 ex[:, cd * 128:(cd + 1) * 128],
                                                 tri_sb[:])
                        exs[h] = ex
                    if pend is not None:
                        av_block(*pend)
                    pend = (j, lo, exs)
                av_block(*pend)

                for h in hs:
                    rc = rcb_p.tile([1, 512], F32, tag="rc")
                    nc.vector.reciprocal_approx_fast(rc[:], pdens[h][0:1, :])
                    bc = rcb_p.tile([128, 512], F32, tag="bc")
                    nc.gpsimd.partition_broadcast(bc[:], rc[:])
                    nc.vector.tensor_tensor(atile[:, h, :], pats[h][:], bc[:],
                                            op=OP.mult)

            # out-proj for the supertile's 4 row blocks
            for c in range(4):
                i = 4 * g + c
                ot = ost_p.tile([128, D], F32, tag="ot")
                for n in range(NOC):
                    po = ps_pat.tile([128, 512], F32, tag="pat",
                                     name=f"po{i}_{n}")
                    for h in range(HPC):
                        nc.tensor.matmul(po[:],
                                         atile[:, h, c * 128:(c + 1) * 128],
                                         wo4[:, h, n, :],
                                         start=(h == 0), stop=(h == HPC - 1))
                    if n % 2 == 0:
                        nc.scalar.copy(ot[:, n * 512:(n + 1) * 512], po[:])
                    else:
                        nc.vector.tensor_copy(ot[:, n * 512:(n + 1) * 512], po[:])
                nc.sync.dma_start(out.ap()[i * 128:(i + 1) * 128, 0:2048],
                                  ot[:, 0:2048])
                nc.sync.dma_start(out.ap()[i * 128:(i + 1) * 128, 2048:4096],
                                  ot[:, 2048:4096])

        # ---------------- main interleaved schedule ------------------------
        load_w_initial()
        dma_block(0)
        for i in range(NB + 1):
            if i == 0:
                load_w_rest()
            if i + 1 < NB:
                dma_block(i + 1)
            if 1 <= i <= 4:
                load_wo(i - 1)
            if i < NB:
                proj_mms(i)
                rms_rope(i)
            if i > 0:
                transp_block(i - 1)
                if (i - 1) % 4 == 3:
                    attn_supertile((i - 1) // 4)


_NC_CACHE = None


def _build():
    global _NC_CACHE
    if _NC_CACHE is None:
        nc = bacc.Bacc("TRN2", target_bir_lowering=False, debug=False)
        with tile.TileContext(nc) as tc:
            _emit(nc, tc)
        nc.compile()
        _NC_CACHE = nc
    return _NC_CACHE


def kernel(x, mask, cos, sin, Wq, Wk, Wv, Wo, qn_w, kn_w):
    x = np.asarray(x, np.float32)
    cos = np.asarray(cos, np.float32)
    sin = np.asarray(sin, np.float32)
    Wq = np.asarray(Wq, np.float32)
    Wk = np.asarray(Wk, np.float32)
    Wv = np.asarray(Wv, np.float32)
    Wo = np.asarray(Wo, np.float32)
    qn_w = np.asarray(qn_w, np.float32)
    kn_w = np.asarray(kn_w, np.float32)

    nc = _build()

    bf = ml_dtypes.bfloat16
    # xt: [NB, 128(d within ktile), DKT*128] blocks of x^T
    xt = np.ascontiguousarray(
        x.T.reshape(DKT, 128, NB, 128).transpose(2, 1, 0, 3)
    ).reshape(NB, 128, DKT * 128).astype(bf)

    # rope tables with rmsnorm sqrt(HD) and q/k norm weights folded in
    sgn = np.concatenate([-np.ones(HD // 2, np.float32),
                          np.ones(HD // 2, np.float32)])
    rt = float(np.sqrt(HD))
    cq = cos * (qn_w * rt)[None, :]
    sq = sin * (sgn * np.roll(qn_w, -(HD // 2)) * rt)[None, :]
    ck = cos * (kn_w * rt)[None, :]
    sk = sin * (sgn * np.roll(kn_w, -(HD // 2)) * rt)[None, :]
    tblp = np.concatenate([np.tile(cq, (1, 4)), np.tile(sq, (1, 4)), ck, sk],
                          axis=1).astype(bf)

    tri_np = np.where(np.arange(128)[:, None] > np.arange(128)[None, :],
                      np.float32(0.0), np.float32(1.0))

    base = dict(
        xt=xt, tbl=tblp, tri01=tri_np,
        ident=np.eye(128, dtype=np.float32).astype(bf),
        onescol=np.ones((128, 1), np.float32).astype(bf),
    )
    in_maps = []
    for cidx in range(NCORES):
        wq_c = Wq[:, cidx * HPC * HD:(cidx + 1) * HPC * HD]
        wq_t = np.ascontiguousarray(
            wq_c.reshape(DKT, 128, HPC * HD).transpose(1, 0, 2)
        ).reshape(128, DKT * HPC * HD).astype(bf)
        wk_c = Wk[:, cidx * HD:(cidx + 1) * HD]
        wv_c = Wv[:, cidx * HD:(cidx + 1) * HD]
        wkv_c = np.concatenate([wk_c, wv_c], axis=1)
        wkv_t = np.ascontiguousarray(
            wkv_c.reshape(DKT, 128, 256).transpose(1, 0, 2)
        ).reshape(128, DKT * 256).astype(bf)
        wo_c = Wo[cidx * HPC * HD:(cidx + 1) * HPC * HD, :]
        wo_t = np.ascontiguousarray(
            wo_c.reshape(HPC, HD, NOC, 512).transpose(1, 0, 2, 3)
        ).reshape(128, HPC * NOC * 512).astype(bf)
        in_maps.append(dict(base, wq=wq_t, wkv=wkv_t, wo=wo_t))

    res = run_bass_kernel_spmd(nc, in_maps, core_ids=list(range(NCORES)))
    acc = res.results[0]["out"].astype(np.float32).copy()
    for r in res.results[1:]:
        acc += r["out"]
    return acc
